# revision 1
# baseline (speedup 1.0000x reference)
"""Trainium2 Bass kernel for nn_ComplexTransformer (complex transformer,
DEPTH=2, B=2, N=1024, DIM=512, HEADS=8, DH=64, FF=2048).

Sharding: the 2048 tokens (B*N flattened) are split 8 ways, 256 tokens per
core; cores 0-3 own batch 0, cores 4-7 own batch 1.  Everything is
token-parallel except attention, which needs full-sequence K/V per batch:
K (transposed, post-rotary) and V (natural) are AllGathered within each
4-core batch group once per layer.  QKV/attention/Wo matmuls run in fp32r
(20-bit float, 11-bit mantissa) at full PE rate; the FF matmuls run in bf16.

Layouts on chip:
  - residual stream xc: natural [128 tok, TB=2, DIM] per complex plane
  - normed activations h_T: transposed [128 dim, KB=4, 256 tok]
  - scores computed transposed [j, i]; softmax denominator via an appended
    ones-column on V (row 64 of the AV psum accumulates sum_j a[j,i])
"""

import os
import sys

# The grading/test process may pin JAX_PLATFORMS=cpu for the reference;
# our executor needs the axon (or neuron) PJRT platform visible.
_jp = os.environ.get("JAX_PLATFORMS")
if _jp is not None and _jp.strip() and "axon" not in _jp:
    os.environ["JAX_PLATFORMS"] = ""

for _p in ("/opt/trn_rl_repo/concourse", "/opt/trn_rl_repo"):
    if _p not in sys.path:
        sys.path.insert(0, _p)

import ml_dtypes
import numpy as np

import concourse.bass as bass
import concourse.bacc as bacc
import concourse.mybir as mybir
import concourse.tile as tile
from concourse.bass_utils import run_bass_kernel_spmd
from concourse.masks import make_identity

F32 = mybir.dt.float32
F32R = mybir.dt.float32r
BF16 = mybir.dt.bfloat16
AF = mybir.ActivationFunctionType
ALU = mybir.AluOpType

# model dims
L = 2
B = 2
N = 1024
DIM = 512
HEADS = 8
DH = 64
INNER = 512
FF = 2048
EPS = 1e-6
SCALE = DH ** -0.5

# sharding dims
NCORES = 8
TOK = 256      # tokens per core
TB = 2         # 128-token blocks per core
KB = 4         # 128-dim blocks of DIM/INNER
JB = 8         # 128-token key blocks per batch (N/128)
OBF = 16       # 128-dim blocks of FF
HP = 4         # head pairs

# combo table: (k plane, q plane, v plane, exp-scale sign)
# rr: qr*kr vr ; ri: qr*(-ki) vr ; ir: qi*kr vi ; ii: qi*(-ki) vi
COMBOS = [(0, 0, 0, 1.0), (1, 0, 0, -1.0), (0, 1, 1, 1.0), (1, 1, 1, -1.0)]


def build_nc(taps=False, unit_gamma=False, zero_mb=False):
    nc = bacc.Bacc("TRN2", target_bir_lowering=False, num_devices=NCORES)

    # ---- I/O ----
    x_in = nc.dram_tensor("x", [2, TB, 128, DIM], F32, kind="ExternalInput")
    # lhsT weights, host layout [L, plane, ob, 128(k part), KB(kb), 128(m)]
    wq = nc.dram_tensor("wq", [L, 2, 4, 128, KB, 128], F32R, kind="ExternalInput")
    wk = nc.dram_tensor("wk", [L, 2, 4, 128, KB, 128], F32R, kind="ExternalInput")
    w1 = nc.dram_tensor("w1", [L, 2, OBF, 128, KB, 128], BF16, kind="ExternalInput")
    # moving weights [L, plane, kb, 128(k part), out]
    wv = nc.dram_tensor("wv", [L, 2, KB, 128, INNER], F32R, kind="ExternalInput")
    wo = nc.dram_tensor("wo", [L, 2, KB, 128, DIM], F32R, kind="ExternalInput")
    w2 = nc.dram_tensor("w2", [L, 2, OBF, 128, DIM], BF16, kind="ExternalInput")
    b1c = nc.dram_tensor("b1c", [L, 2, 128, OBF], F32, kind="ExternalInput")
    b2b = nc.dram_tensor("b2b", [L, 2, 128, DIM], F32, kind="ExternalInput")
    g_at = nc.dram_tensor("g_at", [L, 2, 128, DIM], F32, kind="ExternalInput")
    g_ff = nc.dram_tensor("g_ff", [L, 2, 128, DIM], F32, kind="ExternalInput")
    g_fin = nc.dram_tensor("g_fin", [2, 128, DIM], F32, kind="ExternalInput")
    rotc = nc.dram_tensor("rotc", [128, TOK], F32, kind="ExternalInput")
    rots = nc.dram_tensor("rots", [128, TOK], F32, kind="ExternalInput")
    mbias = nc.dram_tensor("mbias", [128, L], F32, kind="ExternalInput")
    out_d = nc.dram_tensor("out", [2, TB, 128, DIM], F32, kind="ExternalOutput")

    tap_d = {}
    if taps:
        for name, shape, dt_ in [
            ("hT0", [2, 128, KB, TOK], F32R),
            ("qT0", [2, 128, KB, TOK], BF16),
            ("kT0", [2, 128, KB, 1024], BF16),
            ("oT0", [2, 128, KB, TOK], F32R),
            ("xc1", [2, TB, 128, DIM], F32),
            ("xc2", [2, TB, 128, DIM], F32),
        ]:
            tap_d[name] = nc.dram_tensor(name, shape, dt_, kind="ExternalOutput")

    replica_groups = [[0, 1, 2, 3], [4, 5, 6, 7]]
    uid = [0]

    def un(s):
        uid[0] += 1
        return f"{s}{uid[0]}"

    with tile.TileContext(nc) as tc:
        with (
            tc.tile_pool(name="consts", bufs=1) as consts,
            tc.tile_pool(name="xcp", bufs=1) as xcp,
            tc.tile_pool(name="hTp", bufs=1) as hTp,
            tc.tile_pool(name="qTp", bufs=1) as qTp,
            tc.tile_pool(name="kfull", bufs=1) as kfullp,
            tc.tile_pool(name="oTp", bufs=1) as oTp,
            tc.tile_pool(name="gb", bufs=2) as gbp,
            tc.tile_pool(name="wsm", bufs=3) as wsm,      # [128,KB,128] lhsT tiles
            tc.tile_pool(name="wbg", bufs=3) as wbg,      # [128,512] moving tiles
            tc.tile_pool(name="ntmp", bufs=1) as ntmp,
            tc.tile_pool(name="small", bufs=3) as smallp,
            tc.tile_pool(name="vhp", bufs=3) as vhp,
            tc.tile_pool(name="atp", bufs=4) as atp,
            tc.tile_pool(name="ocp", bufs=3) as ocp,
            tc.tile_pool(name="ffp", bufs=1) as ffp,
            tc.tile_pool(name="dram", bufs=2, space="DRAM") as dramp,
        ):
            ident = consts.tile([128, 128], F32)
            make_identity(nc, ident)
            rc_t = consts.tile([128, TOK], F32)
            rs_t = consts.tile([128, TOK], F32)
            nc.sync.dma_start(out=rc_t[:], in_=rotc[:])
            nc.sync.dma_start(out=rs_t[:], in_=rots[:])
            mb_t = consts.tile([128, L], F32)
            nc.sync.dma_start(out=mb_t[:], in_=mbias[:])
            eps_t = consts.tile([128, 1], F32)
            nc.vector.memset(eps_t[:], EPS)
            tiny_t = consts.tile([128, 1], F32)
            nc.vector.memset(tiny_t[:], 1e-30)

            # residual stream
            xc = [xcp.tile([128, TB, DIM], F32, name=f"xc{p}") for p in range(2)]
            for p in range(2):
                for tb in range(TB):
                    nc.sync.dma_start(out=xc[p][:, tb, :], in_=x_in[p, tb])

            def norm_natural(g_dram, h_nat):
                """rmsnorm(xc)*gamma in natural layout into h_nat (2 tiles)."""
                if not unit_gamma:
                    g0 = ntmp.tile([128, DIM], F32, name="g0", bufs=1)
                    g1 = ntmp.tile([128, DIM], F32, name="g1", bufs=1)
                    nc.sync.dma_start(out=g0[:], in_=g_dram[0])
                    nc.sync.dma_start(out=g1[:], in_=g_dram[1])
                for tb in range(TB):
                    sq = ntmp.tile([128, DIM], F32, name="sq")
                    ss0 = ntmp.tile([128, 1], F32, name="ss0")
                    ss1 = ntmp.tile([128, 1], F32, name="ss1")
                    nc.scalar.activation(sq[:], xc[0][:, tb, :], AF.Square,
                                         accum_out=ss0[:])
                    sq2 = ntmp.tile([128, DIM], F32, name="sq2")
                    nc.scalar.activation(sq2[:], xc[1][:, tb, :], AF.Square,
                                         accum_out=ss1[:])
                    s_all = ntmp.tile([128, 1], F32, name="s_all")
                    nc.vector.tensor_add(s_all[:], ss0[:], ss1[:])
                    rms = ntmp.tile([128, 1], F32, name="rms")
                    nc.scalar.activation(rms[:], s_all[:], AF.Sqrt,
                                         scale=1.0 / DIM, bias=eps_t[:])
                    rstd = ntmp.tile([128, 1], F32, name="rstd")
                    nc.vector.reciprocal(rstd[:], rms[:])
                    if unit_gamma:
                        nc.vector.tensor_scalar_mul(h_nat[0][:, tb, :],
                                                    xc[0][:, tb, :], rstd[:])
                        nc.vector.tensor_scalar_mul(h_nat[1][:, tb, :],
                                                    xc[1][:, tb, :], rstd[:])
                        continue
                    # h_r = (xr*gr - xi*gi)*rstd ; h_i = (xr*gi + xi*gr)*rstd
                    t1 = ntmp.tile([128, DIM], F32, name="t1")
                    t2 = ntmp.tile([128, DIM], F32, name="t2")
                    nc.vector.tensor_mul(t1[:], xc[0][:, tb, :], g0[:])
                    nc.vector.scalar_tensor_tensor(
                        t2[:], xc[1][:, tb, :], -1.0, g1[:],
                        op0=ALU.mult, op1=ALU.mult)
                    u = ntmp.tile([128, DIM], F32, name="u")
                    nc.vector.tensor_add(u[:], t1[:], t2[:])
                    nc.vector.tensor_scalar_mul(h_nat[0][:, tb, :], u[:], rstd[:])
                    t3 = ntmp.tile([128, DIM], F32, name="t1")
                    t4 = ntmp.tile([128, DIM], F32, name="t2")
                    nc.vector.tensor_mul(t3[:], xc[0][:, tb, :], g1[:])
                    nc.vector.tensor_mul(t4[:], xc[1][:, tb, :], g0[:])
                    u2 = ntmp.tile([128, DIM], F32, name="u")
                    nc.vector.tensor_add(u2[:], t3[:], t4[:])
                    nc.vector.tensor_scalar_mul(h_nat[1][:, tb, :], u2[:], rstd[:])

            def rmsnorm_transpose(g_dram, dtype, tap=None):
                """rmsnorm(xc)*gamma, transposed to [128 dim, KB, TOK] tiles
                (r, i, and negated i)."""
                hn0 = ntmp.tile([128, TB, DIM], F32, name="hn0", bufs=1)
                hn1 = ntmp.tile([128, TB, DIM], F32, name="hn1", bufs=1)
                h_nat = [hn0, hn1]
                norm_natural(g_dram, h_nat)
                hT_r = hTp.tile([128, KB, TOK], dtype, name="hTr")
                hT_i = hTp.tile([128, KB, TOK], dtype, name="hTi")
                hT_in = hTp.tile([128, KB, TOK], dtype, name="hTn")
                outs = [hT_r, hT_i]
                with tc.tile_pool(name=un("pst"), bufs=2, space="PSUM") as pst:
                    for p in range(2):
                        for tb in range(TB):
                            for kb in range(KB):
                                pt = pst.tile([128, 128], F32, name="pt")
                                nc.tensor.transpose(
                                    pt[:], h_nat[p][:, tb, kb * 128:(kb + 1) * 128],
                                    ident[:])
                                dst = outs[p][:, kb, tb * 128:(tb + 1) * 128]
                                nc.scalar.copy(dst, pt[:])
                                if p == 1:
                                    nc.vector.tensor_scalar_mul(
                                        hT_in[:, kb, tb * 128:(tb + 1) * 128],
                                        pt[:], -1.0)
                if tap is not None:
                    for p in range(2):
                        nc.sync.dma_start(out=tap[p], in_=outs[p][:])
                return hT_r, hT_i, hT_in

            def attn_block(l):
                dotap = taps and l == 0
                hT_r, hT_i, hT_in = rmsnorm_transpose(
                    g_at[l], F32R,
                    tap=[tap_d["hT0"][p] for p in range(2)] if dotap else None)

                # ---- q/k projections (transposed) + rotary ----
                q_T = [qTp.tile([128, KB, TOK], BF16, name=f"qT{p}") for p in range(2)]
                kT_loc = [qTp.tile([128, KB, TOK], BF16, name=f"kTl{p}") for p in range(2)]
                v_loc = ffp.tile([128, TB, 2, INNER], BF16, name="vloc", bufs=1)

                def proj_rot(dst, w_dram, psp):
                    for ob in range(KB):
                        # one weight tile per plane covering all KB k-blocks
                        wt0 = wsm.tile([128, KB, 128], F32R, name="wt0")
                        wt1 = wsm.tile([128, KB, 128], F32R, name="wt1")
                        nc.sync.dma_start(out=wt0[:], in_=w_dram[l, 0, ob])
                        nc.sync.dma_start(out=wt1[:], in_=w_dram[l, 1, ob])
                        ps_r = psp.tile([128, TOK], F32, name="pr")
                        ps_i = psp.tile([128, TOK], F32, name="pi")
                        for ps, terms in (
                            (ps_r, [(wt0, hT_r), (wt1, hT_in)]),
                            (ps_i, [(wt1, hT_r), (wt0, hT_i)]),
                        ):
                            i = 0
                            for kb in range(KB):
                                for wt, act in terms:
                                    nc.tensor.matmul(
                                        ps[:], wt[:, kb, :], act[:, kb, :],
                                        start=(i == 0), stop=(i == 2 * KB - 1))
                                    i += 1
                        # rotary: out_r = pr*c - pi*s; out_i = pr*s + pi*c
                        t1 = smallp.tile([128, TOK], F32, name="r1", bufs=1)
                        t2 = smallp.tile([128, TOK], F32, name="r2", bufs=1)
                        nc.vector.tensor_mul(t1[:], ps_r[:], rc_t[:])
                        nc.vector.scalar_tensor_tensor(
                            t2[:], ps_i[:], -1.0, rs_t[:],
                            op0=ALU.mult, op1=ALU.mult)
                        nc.vector.tensor_add(dst[0][:, ob, :], t1[:], t2[:])
                        t3 = smallp.tile([128, TOK], F32, name="r3", bufs=1)
                        t4 = smallp.tile([128, TOK], F32, name="r4", bufs=1)
                        nc.vector.tensor_mul(t3[:], ps_r[:], rs_t[:])
                        nc.vector.tensor_mul(t4[:], ps_i[:], rc_t[:])
                        nc.vector.tensor_add(dst[1][:, ob, :], t3[:], t4[:])

                with tc.tile_pool(name=un("psp"), bufs=2, space="PSUM") as psp:
                    proj_rot(kT_loc, wk, psp)
                    # ---- AllGather #1: k_T (post-rotary), overlaps v/q proj
                    bounce_k = dramp.tile([2, 512 * 256], BF16, name="bounce_k")
                    gath_k = dramp.tile([4, 2, 512 * 256], BF16, name="gath_k")
                    for p in range(2):
                        bk = bounce_k[p].rearrange("(q x) -> q x", q=128)
                        nc.sync.dma_start(out=bk, in_=kT_loc[p][:].rearrange(
                            "q kb t -> q (kb t)"))
                    nc.gpsimd.collective_compute(
                        "AllGather", ALU.bypass, replica_groups=replica_groups,
                        ins=[bounce_k[:].opt()], outs=[gath_k[:].opt()])
                    # v projection (natural layout): kb-outer accumulation into
                    # 4 live psums so only 2 weight tiles are live at a time
                    psv = [[psp.tile([128, INNER], F32, name="pv", bufs=4)
                            for _ in range(TB)] for _ in range(2)]
                    for kb in range(KB):
                        wv0 = wbg.tile([128, INNER], F32R, name="wtv", bufs=4)
                        wv1 = wbg.tile([128, INNER], F32R, name="wtv", bufs=4)
                        nc.sync.dma_start(out=wv0[:], in_=wv[l, 0, kb])
                        nc.sync.dma_start(out=wv1[:], in_=wv[l, 1, kb])
                        for pl_out in range(2):
                            terms = ([(hT_r, wv0), (hT_in, wv1)] if pl_out == 0
                                     else [(hT_r, wv1), (hT_i, wv0)])
                            for tb in range(TB):
                                i = 0
                                for act, wt in terms:
                                    nc.tensor.matmul(
                                        psv[pl_out][tb][:],
                                        act[:, kb, tb * 128:(tb + 1) * 128],
                                        wt[:],
                                        start=(kb == 0 and i == 0),
                                        stop=(kb == KB - 1 and i == 1))
                                    i += 1
                    for tb in range(TB):
                        for pl_out in range(2):
                            nc.scalar.copy(v_loc[:, tb, pl_out, :],
                                           psv[pl_out][tb][:])

                    # ---- AllGather #2: v (natural layout) ----
                    bounce_v = dramp.tile([2, 512 * 256], BF16, name="bounce_v")
                    gath_v = dramp.tile([4, 2, 512 * 256], BF16, name="gath_v")
                    for p in range(2):
                        bv = bounce_v[p].rearrange("(tb q d) -> tb q d",
                                                   tb=TB, q=128)
                        for tb in range(TB):
                            nc.sync.dma_start(out=bv[tb], in_=v_loc[:, tb, p, :])
                    nc.gpsimd.collective_compute(
                        "AllGather", ALU.bypass, replica_groups=replica_groups,
                        ins=[bounce_v[:].opt()], outs=[gath_v[:].opt()])

                    # q projection + rotary overlaps the collectives
                    proj_rot(q_T, wq, psp)

                if dotap:
                    for p in range(2):
                        nc.sync.dma_start(out=tap_d["qT0"][p], in_=q_T[p][:])

                # ---- load gathered k into [128, 2, KB, 1024] ----
                k_full = kfullp.tile([128, 2, KB, 1024], BF16, name="kfull")
                for src in range(4):
                    for p in range(2):
                        gk = gath_k[src, p].rearrange("(q kb t) -> q kb t",
                                                      q=128, kb=KB)
                        nc.sync.dma_start(
                            out=k_full[:, p, :, src * 256:(src + 1) * 256],
                            in_=gk)
                if dotap:
                    for p in range(2):
                        nc.sync.dma_start(out=tap_d["kT0"][p],
                                          in_=k_full[:, p, :, :])

                # ---- attention ----
                oT_r = oTp.tile([128, KB, TOK], F32R, name="oTr")
                oT_i = oTp.tile([128, KB, TOK], F32R, name="oTi")
                oT_in = oTp.tile([128, KB, TOK], F32R, name="oTn")
                with (
                    tc.tile_pool(name=un("pss"), bufs=4, space="PSUM") as pss,
                    tc.tile_pool(name=un("pso"), bufs=4, space="PSUM") as pso,
                ):
                    for hp in range(HP):
                        vh = []
                        for h01 in range(2):
                            h = 2 * hp + h01
                            vt = vhp.tile([128, JB, 2, 65], BF16, name="vh")
                            nc.vector.memset(vt[:], 1.0)
                            for src in range(4):
                                for half in range(2):
                                    jb = src * 2 + half
                                    for p in range(2):
                                        gv = gath_v[src, p].rearrange(
                                            "(t d) -> t d", d=INNER)
                                        nc.sync.dma_start(
                                            out=vt[:, jb, p, 0:64],
                                            in_=gv[half * 128:(half + 1) * 128,
                                                   h * 64:(h + 1) * 64])
                            vh.append(vt)

                        ocs = [[None, None] for _ in range(4)]
                        for ci, (kpl, qpl, vpl, sgn) in enumerate(COMBOS):
                            ops = [pso.tile([65, TOK], F32, name="op")
                                   for _ in range(2)]
                            for jj in range(JB // 2):
                                # scores for a pair of key-blocks per head, so
                                # one exp covers [128, 512]
                                sp2s = [pss.tile([128, 2, TOK], F32, name="sp")
                                        for _ in range(2)]
                                for sub in range(2):
                                    jb = jj * 2 + sub
                                    for h01 in range(2):
                                        pr = slice(64 * h01, 64 * h01 + 64)
                                        nc.tensor.matmul(
                                            sp2s[h01][:, sub, :],
                                            k_full[pr, kpl, hp,
                                                   jb * 128:(jb + 1) * 128],
                                            q_T[qpl][pr, hp, :],
                                            start=True, stop=True)
                                for h01 in range(2):
                                    at = atp.tile([128, 2, TOK], BF16, name="at")
                                    nc.scalar.activation(at[:], sp2s[h01][:],
                                                         AF.Exp,
                                                         scale=float(sgn * SCALE))
                                    for sub in range(2):
                                        jb = jj * 2 + sub
                                        nc.tensor.matmul(
                                            ops[h01][:], vh[h01][:, jb, vpl, :],
                                            at[:, sub, :],
                                            start=(jb == 0), stop=(jb == JB - 1))
                            for h01 in range(2):
                                rec = smallp.tile([1, TOK], F32, name="rec",
                                                  bufs=4)
                                nc.vector.reciprocal(rec[:], ops[h01][64:65, :])
                                bc = smallp.tile([64, TOK], F32, name="bc",
                                                 bufs=4)
                                nc.gpsimd.partition_broadcast(bc[:], rec[:])
                                oc = ocp.tile([64, TOK], F32, name="oc",
                                              bufs=10)
                                nc.vector.tensor_mul(oc[:],
                                                     ops[h01][0:64, :], bc[:])
                                ocs[ci][h01] = oc
                        # combine at base partition 0 (SB-SB ops must share
                        # base); odd head's result is DMA-shifted into place
                        for h01 in range(2):
                            if h01 == 0:
                                d_r = oT_r[0:64, hp, :]
                                d_i = oT_i[0:64, hp, :]
                                d_in = oT_in[0:64, hp, :]
                            else:
                                st_r = ocp.tile([64, TOK], F32R, name="st_r",
                                                bufs=2)
                                st_i = ocp.tile([64, TOK], F32R, name="st_i",
                                                bufs=2)
                                st_in = ocp.tile([64, TOK], F32R, name="st_in",
                                                 bufs=2)
                                d_r, d_i, d_in = st_r[:], st_i[:], st_in[:]
                            nc.vector.tensor_sub(d_r, ocs[0][h01][:],
                                                 ocs[3][h01][:])
                            nc.vector.tensor_add(d_i, ocs[1][h01][:],
                                                 ocs[2][h01][:])
                            nc.vector.scalar_tensor_tensor(
                                d_in, ocs[1][h01][:], -1.0, ocs[2][h01][:],
                                op0=ALU.mult, op1=ALU.subtract)
                            if h01 == 1:
                                nc.sync.dma_start(out=oT_r[64:128, hp, :],
                                                  in_=st_r[:])
                                nc.sync.dma_start(out=oT_i[64:128, hp, :],
                                                  in_=st_i[:])
                                nc.sync.dma_start(out=oT_in[64:128, hp, :],
                                                  in_=st_in[:])

                if dotap:
                    for p, t in ((0, oT_r), (1, oT_i)):
                        nc.sync.dma_start(out=tap_d["oT0"][p], in_=t[:])

                # ---- Wo projection (natural out) + residual ----
                with tc.tile_pool(name=un("psw"), bufs=4, space="PSUM") as psw:
                    pso_t = [[psw.tile([128, DIM], F32, name="po", bufs=4)
                              for _ in range(TB)] for _ in range(2)]
                    for kb in range(KB):
                        wo0 = wbg.tile([128, DIM], F32R, name="wto", bufs=4)
                        wo1 = wbg.tile([128, DIM], F32R, name="wto", bufs=4)
                        nc.sync.dma_start(out=wo0[:], in_=wo[l, 0, kb])
                        nc.sync.dma_start(out=wo1[:], in_=wo[l, 1, kb])
                        for pl_out in range(2):
                            terms = ([(oT_r, wo0), (oT_in, wo1)] if pl_out == 0
                                     else [(oT_r, wo1), (oT_i, wo0)])
                            for tb in range(TB):
                                i = 0
                                for act, wt in terms:
                                    nc.tensor.matmul(
                                        pso_t[pl_out][tb][:],
                                        act[:, kb, tb * 128:(tb + 1) * 128],
                                        wt[:],
                                        start=(kb == 0 and i == 0),
                                        stop=(kb == KB - 1 and i == 1))
                                    i += 1
                    for pl_out in range(2):
                        for tb in range(TB):
                            nc.vector.tensor_add(xc[pl_out][:, tb, :],
                                                 xc[pl_out][:, tb, :],
                                                 pso_t[pl_out][tb][:])

            def ff_block(l):
                hT_r, hT_i, hT_in = rmsnorm_transpose(g_ff[l], BF16)
                b1t0 = gbp.tile([128, OBF], F32, name="b1t0", bufs=1)
                b1t1 = gbp.tile([128, OBF], F32, name="b1t1", bufs=1)
                b1t = [b1t0, b1t1]
                nc.sync.dma_start(out=b1t0[:], in_=b1c[l, 0])
                nc.sync.dma_start(out=b1t1[:], in_=b1c[l, 1])
                b2t0 = gbp.tile([128, DIM], F32, name="b2t0", bufs=1)
                b2t1 = gbp.tile([128, DIM], F32, name="b2t1", bufs=1)
                b2t = [b2t0, b2t1]
                nc.sync.dma_start(out=b2t0[:], in_=b2b[l, 0])
                nc.sync.dma_start(out=b2t1[:], in_=b2b[l, 1])

                with (
                    tc.tile_pool(name=un("psa"), bufs=4, space="PSUM") as psa,
                    tc.tile_pool(name=un("ps2"), bufs=4, space="PSUM") as ps2,
                ):
                    w2ps = [[ps2.tile([128, DIM], F32, name="w2ps")
                             for _ in range(TB)] for _ in range(2)]
                    for kb in range(OBF):
                        wt0 = wsm.tile([128, KB, 128], BF16, name="f0", bufs=3)
                        wt1 = wsm.tile([128, KB, 128], BF16, name="f1", bufs=3)
                        nc.sync.dma_start(out=wt0[:], in_=w1[l, 0, kb])
                        nc.sync.dma_start(out=wt1[:], in_=w1[l, 1, kb])
                        pA = []
                        for pl_out in range(2):
                            ps = psa.tile([128, TOK], F32, name="pA")
                            terms = ([(wt0, hT_r), (wt1, hT_in)] if pl_out == 0
                                     else [(wt1, hT_r), (wt0, hT_i)])
                            i = 0
                            for kbd in range(KB):
                                for wt, act in terms:
                                    nc.tensor.matmul(ps[:], wt[:, kbd, :],
                                                     act[:, kbd, :],
                                                     start=(i == 0),
                                                     stop=(i == 2 * KB - 1))
                                    i += 1
                            pA.append(ps)
                        # bias add (ACT copy w/ bias), then modrelu
                        a_r = ffp.tile([128, TOK], F32, name="a_r", bufs=2)
                        a_i = ffp.tile([128, TOK], F32, name="a_i", bufs=2)
                        nc.scalar.activation(a_r[:], pA[0][:], AF.Identity,
                                             bias=b1t[0][:, kb:kb + 1])
                        nc.scalar.activation(a_i[:], pA[1][:], AF.Identity,
                                             bias=b1t[1][:, kb:kb + 1])
                        sq_r = ffp.tile([128, TOK], F32, name="sq_r", bufs=1)
                        sq_i = ffp.tile([128, TOK], F32, name="sq_i", bufs=1)
                        nc.vector.tensor_mul(sq_r[:], a_r[:], a_r[:])
                        nc.vector.tensor_mul(sq_i[:], a_i[:], a_i[:])
                        m2 = ffp.tile([128, TOK], F32, name="m2", bufs=2)
                        nc.vector.tensor_add(m2[:], sq_r[:], sq_i[:])
                        fac = ffp.tile([128, TOK], F32, name="fac", bufs=2)
                        if zero_mb:
                            # factor = relu(mag)^2/mag = mag
                            nc.scalar.activation(fac[:], m2[:], AF.Sqrt)
                        else:
                            mag = ffp.tile([128, TOK], F32, name="mag", bufs=1)
                            nc.scalar.activation(mag[:], m2[:], AF.Sqrt,
                                                 bias=tiny_t[:])
                            rel = ffp.tile([128, TOK], F32, name="rel", bufs=1)
                            nc.scalar.activation(rel[:], mag[:], AF.Relu,
                                                 bias=mb_t[:, l:l + 1])
                            rel2 = ffp.tile([128, TOK], F32, name="rel2", bufs=1)
                            nc.vector.tensor_mul(rel2[:], rel[:], rel[:])
                            rmag = ffp.tile([128, TOK], F32, name="rmag", bufs=1)
                            nc.vector.reciprocal(rmag[:], mag[:])
                            nc.vector.tensor_mul(fac[:], rel2[:], rmag[:])
                        ap_r = ffp.tile([128, TOK], BF16, name="ap_r", bufs=2)
                        ap_i = ffp.tile([128, TOK], BF16, name="ap_i", bufs=2)
                        ap_in = ffp.tile([128, TOK], BF16, name="ap_in", bufs=2)
                        nc.vector.tensor_mul(ap_r[:], a_r[:], fac[:])
                        nc.vector.tensor_mul(ap_i[:], a_i[:], fac[:])
                        nc.vector.scalar_tensor_tensor(
                            ap_in[:], a_i[:], -1.0, fac[:],
                            op0=ALU.mult, op1=ALU.mult)
                        # W2 accumulation
                        w2t0 = wbg.tile([128, DIM], BF16, name="w2t0", bufs=3)
                        w2t1 = wbg.tile([128, DIM], BF16, name="w2t1", bufs=3)
                        nc.sync.dma_start(out=w2t0[:], in_=w2[l, 0, kb])
                        nc.sync.dma_start(out=w2t1[:], in_=w2[l, 1, kb])
                        for pl_out in range(2):
                            terms = ([(ap_r, w2t0), (ap_in, w2t1)] if pl_out == 0
                                     else [(ap_r, w2t1), (ap_i, w2t0)])
                            for tb in range(TB):
                                i2 = 0
                                for act, wt in terms:
                                    nc.tensor.matmul(
                                        w2ps[pl_out][tb][:],
                                        act[:, tb * 128:(tb + 1) * 128], wt[:],
                                        start=(kb == 0 and i2 == 0),
                                        stop=(kb == OBF - 1 and i2 == 1))
                                    i2 += 1
                    # add bias + residual
                    for pl_out in range(2):
                        for tb in range(TB):
                            nc.vector.tensor_add(xc[pl_out][:, tb, :],
                                                 xc[pl_out][:, tb, :],
                                                 w2ps[pl_out][tb][:])
                            nc.vector.tensor_add(xc[pl_out][:, tb, :],
                                                 xc[pl_out][:, tb, :],
                                                 b2t[pl_out][:])

            for l in range(L):
                attn_block(l)
                if taps and l == 0:
                    for p in range(2):
                        for tb in range(TB):
                            nc.sync.dma_start(out=tap_d["xc1"][p, tb],
                                              in_=xc[p][:, tb, :])
                ff_block(l)
                if taps and l == 0:
                    for p in range(2):
                        for tb in range(TB):
                            nc.sync.dma_start(out=tap_d["xc2"][p, tb],
                                              in_=xc[p][:, tb, :])

            # ---- final norm + output ----
            fo0 = ntmp.tile([128, TB, DIM], F32, name="hn0", bufs=1)
            fo1 = ntmp.tile([128, TB, DIM], F32, name="hn1", bufs=1)
            norm_natural(g_fin, [fo0, fo1])
            for p_out, t in ((0, fo0), (1, fo1)):
                for tb in range(TB):
                    nc.sync.dma_start(out=out_d[p_out, tb], in_=t[:, tb, :])

    nc.compile()
    return nc


# ---------------------------------------------------------------------------
# host side: shard, run, unshard
# ---------------------------------------------------------------------------

def _round_fp32r(a):
    u = np.ascontiguousarray(a, dtype=np.float32).view(np.uint32)
    u = ((u.astype(np.uint64) + 0x800) & 0xFFFFF000).astype(np.uint32)
    return u.view(np.float32)


def _prep_shared(Wq, Wkv, Wo, W1, b1, W2, b2, gamma_attn, gamma_ff, mod_bias,
                 gamma_final):
    """Host-side marshalling of the weight tensors (identical on all cores)."""
    def lp(w):  # [L, ..., 2] -> [L, 2, ...]
        return np.moveaxis(np.moveaxis(w, -1, 0), 0, 1)

    sh = {}
    wq_p = lp(Wq)                       # [L, 2, DIM, INNER]
    wk_p = lp(Wkv[:, :, :INNER, :])
    wv_p = lp(Wkv[:, :, INNER:, :])
    wo_p = lp(Wo)                       # [L, 2, INNER, DIM]
    w1_p = lp(W1)                       # [L, 2, DIM, FF]
    w2_p = lp(W2)                       # [L, 2, FF, DIM]

    def lhst(w, nob):                   # -> [L, 2, nob, 128, KB, 128]
        lw = w.reshape(L, 2, KB, 128, nob, 128)
        return np.ascontiguousarray(lw.transpose(0, 1, 4, 3, 2, 5))

    sh["wq"] = _round_fp32r(lhst(wq_p, 4))
    sh["wk"] = _round_fp32r(lhst(wk_p, 4))
    sh["w1"] = lhst(w1_p, OBF).astype(ml_dtypes.bfloat16)
    sh["wv"] = _round_fp32r(
        np.ascontiguousarray(wv_p.reshape(L, 2, KB, 128, INNER)))
    sh["wo"] = _round_fp32r(
        np.ascontiguousarray(wo_p.reshape(L, 2, KB, 128, DIM)))
    sh["w2"] = np.ascontiguousarray(
        w2_p.reshape(L, 2, OBF, 128, DIM)).astype(ml_dtypes.bfloat16)

    b1_p = lp(b1)                       # [L, 2, FF]
    sh["b1c"] = np.ascontiguousarray(
        b1_p.reshape(L, 2, OBF, 128).transpose(0, 1, 3, 2))
    b2_p = lp(b2)                       # [L, 2, DIM]
    sh["b2b"] = np.ascontiguousarray(
        np.broadcast_to(b2_p[:, :, None, :], (L, 2, 128, DIM)))
    ga = lp(gamma_attn)                 # [L, 2, DIM]
    sh["g_at"] = np.ascontiguousarray(
        np.broadcast_to(ga[:, :, None, :], (L, 2, 128, DIM)))
    gf = lp(gamma_ff)
    sh["g_ff"] = np.ascontiguousarray(
        np.broadcast_to(gf[:, :, None, :], (L, 2, 128, DIM)))
    gfin = np.moveaxis(gamma_final, -1, 0)      # [2, DIM]
    sh["g_fin"] = np.ascontiguousarray(
        np.broadcast_to(gfin[:, None, :], (2, 128, DIM)))
    sh["mbias"] = np.ascontiguousarray(
        np.broadcast_to(mod_bias[None, :], (128, L)).astype(np.float32))
    return sh


def _rot_tables(core):
    """cos/sin tables [128, TOK] for this core's token positions."""
    inv_freq = 1.0 / (10000.0 ** (np.arange(DH, dtype=np.float64) / DH))
    pos = (core % 4) * TOK + np.arange(TOK, dtype=np.float64)
    dh_idx = np.arange(128) % DH
    freqs = pos[None, :] * inv_freq[dh_idx][:, None]    # [128, TOK]
    return (np.cos(freqs).astype(np.float32),
            np.sin(freqs).astype(np.float32))


_NC_CACHE = {}


def get_nc(taps, unit_gamma, zero_mb):
    key = (taps, unit_gamma, zero_mb)
    if key not in _NC_CACHE:
        _NC_CACHE[key] = build_nc(taps=taps, unit_gamma=unit_gamma,
                                  zero_mb=zero_mb)
    return _NC_CACHE[key]


def make_in_maps(x, gamma_attn, Wq, Wkv, Wo, gamma_ff, W1, b1, mod_bias, W2,
                 b2, gamma_final):
    x = np.asarray(x, dtype=np.float32)
    sh = _prep_shared(np.asarray(Wq, np.float32), np.asarray(Wkv, np.float32),
                      np.asarray(Wo, np.float32), np.asarray(W1, np.float32),
                      np.asarray(b1, np.float32), np.asarray(W2, np.float32),
                      np.asarray(b2, np.float32),
                      np.asarray(gamma_attn, np.float32),
                      np.asarray(gamma_ff, np.float32),
                      np.asarray(mod_bias, np.float32),
                      np.asarray(gamma_final, np.float32))
    xf = x.reshape(B * N, DIM, 2)
    in_maps = []
    for core in range(NCORES):
        tok = xf[core * TOK:(core + 1) * TOK]           # [TOK, DIM, 2]
        xs = np.ascontiguousarray(
            tok.transpose(2, 0, 1).reshape(2, TB, 128, DIM))
        rc, rs = _rot_tables(core)
        m = dict(sh)
        m["x"] = xs
        m["rotc"] = rc
        m["rots"] = rs
        in_maps.append(m)
    return in_maps


def _flags(gamma_attn, gamma_ff, gamma_final, mod_bias):
    def unit(g):
        g = np.asarray(g, np.float32)
        return bool(np.all(g[..., 0] == 1.0) and np.all(g[..., 1] == 0.0))

    unit_gamma = unit(gamma_attn) and unit(gamma_ff) and unit(gamma_final)
    zero_mb = bool(np.all(np.asarray(mod_bias) == 0.0))
    return unit_gamma, zero_mb


def kernel(x, gamma_attn, Wq, Wkv, Wo, gamma_ff, W1, b1, mod_bias, W2, b2,
           gamma_final):
    unit_gamma, zero_mb = _flags(gamma_attn, gamma_ff, gamma_final, mod_bias)
    nc = get_nc(False, unit_gamma, zero_mb)
    in_maps = make_in_maps(x, gamma_attn, Wq, Wkv, Wo, gamma_ff, W1, b1,
                           mod_bias, W2, b2, gamma_final)
    res = run_bass_kernel_spmd(nc, in_maps, core_ids=list(range(NCORES)))
    outs = []
    for core in range(NCORES):
        o = res.results[core]["out"]                    # [2, TB, 128, DIM]
        o = o.reshape(2, TOK, DIM).transpose(1, 2, 0)   # [TOK, DIM, 2]
        outs.append(o)
    full = np.concatenate(outs, axis=0).reshape(B, N, DIM, 2)
    return np.ascontiguousarray(full.astype(np.float32))



# revision 12
# speedup vs baseline: 1.6201x; 1.6201x over previous
"""Trainium2 Bass kernel for nn_ComplexTransformer (complex transformer,
DEPTH=2, B=2, N=1024, DIM=512, HEADS=8, DH=64, FF=2048).

Sharding: 2048 tokens (B*N) split 8 ways, 256 tokens/core; cores 0-3 own
batch 0, cores 4-7 batch 1.  Token-parallel everywhere except attention,
which AllGathers post-rotary K and V (fp8 payload, one merged collective
per head-half per layer -> 2 collectives/layer, pipelined against
attention on the first head-half).

All projection matmuls run fp8e4m3 with DoubleRow perf mode: complex
matmuls are "term-paired" -- the two real terms of each output plane form
the DoubleRow pair, with the negation folded into host-prepared weights:
  out_r = Wr.h_r + (-Wi).h_i ;  out_i = Wi.h_r + Wr.h_i
Weights are scaled x16 (x4 for W1) on the host for fp8 range; the inverse
scales fold into exp scale / residual adds / the modrelu sqrt.

Attention: scores fp8 (plain matmul), exp on ACT into fp8 "at" tiles
([128,4,256] batched over 4 key-blocks), AV as DoubleRow over key-block
pairs with an appended ones-column for the softmax denominator.
"""

import os
import sys

_jp = os.environ.get("JAX_PLATFORMS")
if _jp is not None and _jp.strip() and "axon" not in _jp:
    os.environ["JAX_PLATFORMS"] = ""

for _p in ("/opt/trn_rl_repo/concourse", "/opt/trn_rl_repo"):
    if _p not in sys.path:
        sys.path.insert(0, _p)

import ml_dtypes
import numpy as np

import concourse.bass as bass
import concourse.bacc as bacc
import concourse.mybir as mybir
import concourse.tile as tile
from concourse.bass_utils import run_bass_kernel_spmd
from concourse.masks import make_identity

F32 = mybir.dt.float32
BF16 = mybir.dt.bfloat16
FP8 = mybir.dt.float8e4
AF = mybir.ActivationFunctionType
ALU = mybir.AluOpType
DR = mybir.MatmulPerfMode.DoubleRow

# model dims
L = 2
B = 2
N = 1024
DIM = 512
HEADS = 8
DH = 64
INNER = 512
FF = 2048
EPS = 1e-6
SCALE = DH ** -0.5

# sharding dims
NCORES = 8
TOK = 256      # tokens per core
TB = 2         # 128-token blocks per core
KB = 4         # 128-dim blocks of DIM/INNER
JB = 8         # 128-token key blocks per batch (N/128)
OBF = 16       # 128-dim blocks of FF
HP = 4         # head pairs

WS = 16.0      # fp8 weight scale (wq/wk/wv/wo/w2)
WS1 = 16.0     # fp8 weight scale for W1
LAM = 32.0     # fp8 scale of modrelu output ap = a*mag
VSLOT = 80     # per-head slot width in v_all (64 v + 1 ones + 15 pad)

# combo table: (k plane, q plane, v plane, exp-scale sign)
# rr: qr*kr vr ; ri: qr*(-ki) vr ; ir: qi*kr vi ; ii: qi*(-ki) vi
COMBOS = [(0, 0, 0, 1.0), (1, 0, 0, -1.0), (0, 1, 1, 1.0), (1, 1, 1, -1.0)]


def build_nc(taps=False, unit_gamma=False, zero_mb=False):
    nc = bacc.Bacc("TRN2", target_bir_lowering=False, num_devices=NCORES)

    # ---- I/O ----
    x_in = nc.dram_tensor("x", [2, TB, 128, DIM], F32, kind="ExternalInput")
    # stationary term-paired weights [L, (q|k), outpl, ob, 128k, KB, 2, 128m]
    wqk = nc.dram_tensor("wqk", [L, 2, 2, 4, 128, KB, 2, 128], FP8,
                         kind="ExternalInput")
    w1 = nc.dram_tensor("w1", [L, 2, 128, OBF, KB, 2, 128], FP8,
                        kind="ExternalInput")
    # moving term-paired weights [L, outpl, 128k, kc, 2, OUT]
    wv = nc.dram_tensor("wv", [L, 2, 128, KB, 2, INNER], FP8,
                        kind="ExternalInput")
    wo = nc.dram_tensor("wo", [L, 2, 128, HP, 2, DIM], FP8,
                        kind="ExternalInput")
    w2 = nc.dram_tensor("w2", [L, 2, 128, OBF, 2, DIM], FP8,
                        kind="ExternalInput")
    b1c = nc.dram_tensor("b1c", [L, 2, 128, OBF], F32, kind="ExternalInput")
    b2b = nc.dram_tensor("b2b", [L, 2, 128, DIM], F32, kind="ExternalInput")
    g_at = nc.dram_tensor("g_at", [L, 2, 128, DIM], F32, kind="ExternalInput")
    g_ff = nc.dram_tensor("g_ff", [L, 2, 128, DIM], F32, kind="ExternalInput")
    g_fin = nc.dram_tensor("g_fin", [2, 128, DIM], F32, kind="ExternalInput")
    rotc = nc.dram_tensor("rotc", [128, TOK], F32, kind="ExternalInput")
    rots = nc.dram_tensor("rots", [128, TOK], F32, kind="ExternalInput")
    mbias = nc.dram_tensor("mbias", [128, L], F32, kind="ExternalInput")
    out_d = nc.dram_tensor("out", [2, TB, 128, DIM], F32, kind="ExternalOutput")

    tap_d = {}
    if taps:
        for name, shape, dt_ in [
            ("hT0", [128, 2, KB, TOK], FP8),
            ("qT0", [128, 2, HP, TOK], FP8),
            ("kT0", [128, KB, 2, 1024], FP8),
            ("vA0", [128, 2, JB, 2, 4 * VSLOT], FP8),
            ("oT0", [128, 2, HP, TOK], FP8),
            ("xc1", [2, TB, 128, DIM], F32),
            ("xc2", [2, TB, 128, DIM], F32),
        ]:
            tap_d[name] = nc.dram_tensor(name, shape, dt_, kind="ExternalOutput")

    replica_groups = [[0, 1, 2, 3], [4, 5, 6, 7]]
    uid = [0]

    def un(s):
        uid[0] += 1
        return f"{s}{uid[0]}"

    from contextlib import ExitStack

    with tile.TileContext(nc) as tc, ExitStack() as _es:
        def pool(name, bufs, space="SBUF"):
            return _es.enter_context(
                tc.tile_pool(name=name, bufs=bufs, space=space))

        consts = pool("consts", 1)
        xcp = pool("xcp", 1)
        hTp = pool("hTp", 1)
        qTp = pool("qTp", 1)
        kfullp = pool("kfull", 1)
        vallp = pool("vall", 1)
        oTp = pool("oTp", 1)
        ap2p = pool("ap2p", 1)
        gbp = pool("gb", 2)
        wsm = pool("wsm", 6)       # small lhsT weights
        wmv = pool("wmv", 4)       # wv/wo moving tiles
        wbig = pool("wbig", 2)     # w1 big tiles
        wbg2 = pool("wbg2", 2)     # w2 big tiles
        ntmp = pool("ntmp", 1)
        smallp = pool("small", 4)
        atp = pool("atp", 4)
        ocp = pool("ocp", 4)
        ffp = pool("ffp", 2)
        dramp = pool("dram", 2, space="DRAM")
        if True:
            ident = consts.tile([128, 128], BF16)
            make_identity(nc, ident)
            rc_t = consts.tile([128, TOK], F32)
            rs_t = consts.tile([128, TOK], F32)
            nc.sync.dma_start(out=rc_t[:], in_=rotc[:])
            nc.sync.dma_start(out=rs_t[:], in_=rots[:])
            mb_t = consts.tile([128, L], F32)
            nc.sync.dma_start(out=mb_t[:], in_=mbias[:])
            eps_t = consts.tile([128, 1], F32)
            nc.vector.memset(eps_t[:], EPS)
            tiny_t = consts.tile([128, 1], F32)
            nc.vector.memset(tiny_t[:], 1e-30)

            # residual stream
            xc = [xcp.tile([128, TB, DIM], F32, name=f"xc{p}") for p in range(2)]
            for p in range(2):
                for tb in range(TB):
                    nc.sync.dma_start(out=xc[p][:, tb, :], in_=x_in[p, tb])

            # gathered K (post-rotary) and V for the whole batch
            k_full = kfullp.tile([128, KB, 2, 1024], FP8, name="kfull")
            v_all = vallp.tile([128, 2, JB, 2, 4 * VSLOT], FP8, name="vall")
            va6 = v_all[:].rearrange("q h j p (s c) -> q h j p s c", s=4)

            def norm_natural(g_dram, h_nat, dtype):
                """rmsnorm(xc)*gamma in natural layout into h_nat tiles."""
                if not unit_gamma:
                    g0 = ntmp.tile([128, DIM], F32, name="g0", bufs=1)
                    g1 = ntmp.tile([128, DIM], F32, name="g1", bufs=1)
                    nc.sync.dma_start(out=g0[:], in_=g_dram[0])
                    nc.sync.dma_start(out=g1[:], in_=g_dram[1])
                for tb in range(TB):
                    sq = ntmp.tile([128, DIM], F32, name="sq")
                    ss0 = ntmp.tile([128, 1], F32, name="ss0")
                    ss1 = ntmp.tile([128, 1], F32, name="ss1")
                    nc.scalar.activation(sq[:], xc[0][:, tb, :], AF.Square,
                                         accum_out=ss0[:])
                    sq2 = ntmp.tile([128, DIM], F32, name="sq2")
                    nc.scalar.activation(sq2[:], xc[1][:, tb, :], AF.Square,
                                         accum_out=ss1[:])
                    s_all = ntmp.tile([128, 1], F32, name="s_all")
                    nc.vector.tensor_add(s_all[:], ss0[:], ss1[:])
                    rms = ntmp.tile([128, 1], F32, name="rms")
                    nc.scalar.activation(rms[:], s_all[:], AF.Sqrt,
                                         scale=1.0 / DIM, bias=eps_t[:])
                    rstd = ntmp.tile([128, 1], F32, name="rstd")
                    nc.vector.reciprocal(rstd[:], rms[:])
                    if unit_gamma:
                        nc.vector.tensor_scalar_mul(h_nat[0][:, tb, :],
                                                    xc[0][:, tb, :], rstd[:])
                        nc.vector.tensor_scalar_mul(h_nat[1][:, tb, :],
                                                    xc[1][:, tb, :], rstd[:])
                        continue
                    # h_r = (xr*gr - xi*gi)*rstd ; h_i = (xr*gi + xi*gr)*rstd
                    t1 = ntmp.tile([128, DIM], F32, name="t1")
                    t2 = ntmp.tile([128, DIM], F32, name="t2")
                    nc.vector.tensor_mul(t1[:], xc[0][:, tb, :], g0[:])
                    nc.vector.scalar_tensor_tensor(
                        t2[:], xc[1][:, tb, :], -1.0, g1[:],
                        op0=ALU.mult, op1=ALU.mult)
                    u = ntmp.tile([128, DIM], F32, name="u")
                    nc.vector.tensor_add(u[:], t1[:], t2[:])
                    nc.vector.tensor_scalar_mul(h_nat[0][:, tb, :], u[:], rstd[:])
                    t3 = ntmp.tile([128, DIM], F32, name="t1")
                    t4 = ntmp.tile([128, DIM], F32, name="t2")
                    nc.vector.tensor_mul(t3[:], xc[0][:, tb, :], g1[:])
                    nc.vector.tensor_mul(t4[:], xc[1][:, tb, :], g0[:])
                    u2 = ntmp.tile([128, DIM], F32, name="u")
                    nc.vector.tensor_add(u2[:], t3[:], t4[:])
                    nc.vector.tensor_scalar_mul(h_nat[1][:, tb, :], u2[:], rstd[:])

            def rmsnorm_T_fp8(g_dram, tap=None):
                """rmsnorm(xc), transposed into hT [128, 2pl, KB, TOK] fp8."""
                hn0 = ntmp.tile([128, TB, DIM], BF16, name="hn0", bufs=1)
                hn1 = ntmp.tile([128, TB, DIM], BF16, name="hn1", bufs=1)
                h_nat = [hn0, hn1]
                norm_natural(g_dram, h_nat, BF16)
                hT = hTp.tile([128, 2, KB, TOK], FP8, name="hT")
                with tc.tile_pool(name=un("pst"), bufs=2, space="PSUM") as pst:
                    for p in range(2):
                        for tb in range(TB):
                            pt = pst.tile([128, KB, 128], BF16, name="pt")
                            for kb in range(KB):
                                nc.tensor.transpose(
                                    pt[:, kb, :],
                                    h_nat[p][:, tb, kb * 128:(kb + 1) * 128],
                                    ident[:])
                            nc.vector.tensor_copy(
                                hT[:, p, :, tb * 128:(tb + 1) * 128], pt[:])
                if tap is not None:
                    nc.sync.dma_start(out=tap, in_=hT[:])
                return hT

            def attn_block(l, dotap):
                hT = rmsnorm_T_fp8(g_at[l],
                                   tap=tap_d["hT0"][:] if dotap else None)

                qT = qTp.tile([128, 2, HP, TOK], FP8, name="qT")
                kT = qTp.tile([128, KB, 2, TOK], FP8, name="kT")
                v_loc = qTp.tile([128, 2, TB, 2, 4 * VSLOT], FP8, name="vloc")
                vl6 = v_loc[:].rearrange("q h t p (s c) -> q h t p s c", s=4)
                nc.vector.memset(vl6[:, :, :, :, :, 64:VSLOT], 0.0)
                nc.vector.memset(vl6[:, :, :, :, :, 64:65], 1.0)

                def proj_rot(dst, qk, psp, kb_major=False):
                    """q/k projection + rotary for all 4 obs into dst."""
                    for ob in range(KB):
                        ps = []
                        for opl in range(2):
                            wt = wsm.tile([128, KB, 2, 128], FP8, name="wt")
                            nc.sync.dma_start(out=wt[:], in_=wqk[l, qk, opl, ob])
                            p_ = psp.tile([128, TOK], F32, name="pq")
                            for kb in range(KB):
                                nc.tensor.matmul(
                                    p_[:], wt[:, kb, :, :], hT[:, :, kb, :],
                                    start=(kb == 0), stop=(kb == KB - 1),
                                    perf_mode=DR)
                            ps.append(p_)
                        # rotary: out_r = pr*c - pi*s; out_i = pr*s + pi*c
                        t1 = smallp.tile([128, TOK], F32, name="r1", bufs=2)
                        t2 = smallp.tile([128, TOK], F32, name="r2", bufs=2)
                        nc.vector.tensor_mul(t1[:], ps[0][:], rc_t[:])
                        nc.vector.scalar_tensor_tensor(
                            t2[:], ps[1][:], -1.0, rs_t[:],
                            op0=ALU.mult, op1=ALU.mult)
                        d0 = dst[:, ob, 0, :] if kb_major else dst[:, 0, ob, :]
                        d1 = dst[:, ob, 1, :] if kb_major else dst[:, 1, ob, :]
                        nc.vector.tensor_add(d0, t1[:], t2[:])
                        t3 = smallp.tile([128, TOK], F32, name="r3", bufs=2)
                        t4 = smallp.tile([128, TOK], F32, name="r4", bufs=2)
                        nc.vector.tensor_mul(t3[:], ps[0][:], rs_t[:])
                        nc.vector.tensor_mul(t4[:], ps[1][:], rc_t[:])
                        nc.vector.tensor_add(d1, t3[:], t4[:])

                def v_proj_half(half, psp):
                    """v projection for inner cols half*256:(half+1)*256."""
                    cs = slice(half * 256, (half + 1) * 256)
                    for opl in range(2):
                        wvt = wmv.tile([128, KB, 2, 256], FP8, name="wvt")
                        nc.sync.dma_start(out=wvt[:], in_=wv[l, opl, :, :, :, cs])
                        for tb in range(TB):
                            p_ = psp.tile([128, 256], F32, name="pv")
                            for kb in range(KB):
                                nc.tensor.matmul(
                                    p_[:],
                                    hT[:, :, kb, tb * 128:(tb + 1) * 128],
                                    wvt[:, kb, :, :],
                                    start=(kb == 0), stop=(kb == KB - 1),
                                    perf_mode=DR)
                            pv4 = p_[:].rearrange("q (s c) -> q s c", s=4)
                            nc.vector.tensor_copy(
                                vl6[:, half, tb, opl, :, 0:64], pv4)

                gath = []
                with tc.tile_pool(name=un("psp"), bufs=4, space="PSUM") as psp:
                    # k proj (all obs; halves kb 0-1 / 2-3), v halves, bounce
                    proj_rot(kT, 1, psp, kb_major=True)
                    for half in range(2):
                        v_proj_half(half, psp)
                        bounce = dramp.tile([128, 2304], FP8,
                                            name=f"bounce{half}")
                        gt = dramp.tile([4, 128, 2304], FP8, name=f"gath{half}")
                        nc.sync.dma_start(
                            out=bounce[:, 0:1024],
                            in_=kT[:, half * 2:half * 2 + 2, :, :])
                        nc.sync.dma_start(
                            out=bounce[:, 1024:2304],
                            in_=v_loc[:, half])
                        nc.gpsimd.collective_compute(
                            "AllGather", ALU.bypass,
                            replica_groups=replica_groups,
                            ins=[bounce[:].opt()], outs=[gt[:].opt()])
                        gath.append(gt)
                    # q projection overlaps the collectives
                    proj_rot(qT, 0, psp)

                if dotap:
                    nc.sync.dma_start(out=tap_d["qT0"][:], in_=qT[:])

                def land_half(half):
                    gt = gath[half]
                    for s_ in range(4):
                        gk = gt[s_, :, 0:1024].rearrange(
                            "q (k p t) -> q k p t", k=2, p=2)
                        nc.sync.dma_start(
                            out=k_full[:, half * 2:half * 2 + 2, :,
                                       s_ * 256:(s_ + 1) * 256],
                            in_=gk)
                        nc.sync.dma_start(
                            out=v_all[:, half, s_ * 2:s_ * 2 + 2, :, :],
                            in_=gt[s_, :, 1024:2304])

                oT = oTp.tile([128, 2, HP, TOK], FP8, name="oT")
                with (
                    tc.tile_pool(name=un("pss"), bufs=2, space="PSUM") as pss,
                    tc.tile_pool(name=un("psa"), bufs=2, space="PSUM") as psa,
                ):
                    for hp in range(HP):
                        if hp % 2 == 0:
                            land_half(hp // 2)
                        for h01 in range(2):
                            pr = slice(64 * h01, 64 * h01 + 64)
                            slot = 2 * hp + h01
                            av = psa.tile([65, 4, TOK], F32, name="av")
                            for ci, (kpl, qpl, vpl, sgn) in enumerate(COMBOS):
                                for hj in range(2):
                                    sp = pss.tile([128, 4, TOK], F32, name="sp")
                                    for q4 in range(4):
                                        jb = hj * 4 + q4
                                        nc.tensor.matmul(
                                            sp[:, q4, :],
                                            k_full[pr, hp, kpl,
                                                   jb * 128:(jb + 1) * 128],
                                            qT[pr, qpl, hp, :],
                                            start=True, stop=True)
                                    at = atp.tile([128, 4, TOK], FP8, name="at")
                                    nc.scalar.activation(
                                        at[:], sp[:], AF.Exp,
                                        scale=float(sgn * SCALE / (WS * WS)))
                                    for jq in range(2):
                                        jj = hj * 2 + jq
                                        nc.tensor.matmul(
                                            av[:, ci, :],
                                            va6[:, hp // 2,
                                                2 * jj:2 * jj + 2, vpl,
                                                2 * (hp % 2) + h01, 0:65],
                                            at[:, 2 * jq:2 * jq + 2, :],
                                            start=(jj == 0), stop=(jj == 3),
                                            perf_mode=DR)
                            # combine: oc = av/denom ; o_r = oc0-oc3,
                            # o_i = oc1+oc2
                            rec = smallp.tile([1, 4, TOK], F32, name="rec", bufs=2)
                            nc.vector.reciprocal(rec[:], av[64:65, :, :])
                            bc = smallp.tile([64, 4, TOK], F32, name="bc", bufs=2)
                            nc.gpsimd.partition_broadcast(bc[:], rec[:])
                            oc = ocp.tile([64, 4, TOK], F32, name="oc", bufs=2)
                            nc.vector.tensor_mul(oc[:], av[0:64, :, :], bc[:])
                            if h01 == 0:
                                d_r = oT[0:64, 0, hp, :]
                                d_i = oT[0:64, 1, hp, :]
                            else:
                                st = ocp.tile([64, 2, TOK], FP8, name="st", bufs=2)
                                d_r = st[:, 0, :]
                                d_i = st[:, 1, :]
                            nc.vector.tensor_sub(d_r, oc[:, 0, :], oc[:, 3, :])
                            nc.vector.tensor_add(d_i, oc[:, 1, :], oc[:, 2, :])
                            if h01 == 1:
                                nc.sync.dma_start(out=oT[64:128, :, hp, :],
                                                  in_=st[:])

                if dotap:
                    nc.sync.dma_start(out=tap_d["kT0"][:], in_=k_full[:])
                    nc.sync.dma_start(out=tap_d["vA0"][:], in_=v_all[:])
                    nc.sync.dma_start(out=tap_d["oT0"][:], in_=oT[:])

                # ---- Wo projection + residual (psum = 256x true) ----
                with tc.tile_pool(name=un("psw"), bufs=4, space="PSUM") as psw:
                    for opl in range(2):
                        wot = wmv.tile([128, HP, 2, DIM], FP8, name="wot", bufs=2)
                        nc.sync.dma_start(out=wot[:], in_=wo[l, opl])
                        for tb in range(TB):
                            p_ = psw.tile([128, DIM], F32, name="po")
                            for hp in range(HP):
                                nc.tensor.matmul(
                                    p_[:],
                                    oT[:, :, hp, tb * 128:(tb + 1) * 128],
                                    wot[:, hp, :, :],
                                    start=(hp == 0), stop=(hp == HP - 1),
                                    perf_mode=DR)
                            nc.vector.scalar_tensor_tensor(
                                xc[opl][:, tb, :], p_[:], 1.0 / (WS * WS),
                                xc[opl][:, tb, :],
                                op0=ALU.mult, op1=ALU.add)

            def ff_block(l, dotap):
                hT = rmsnorm_T_fp8(g_ff[l])
                b1t = gbp.tile([128, 2, OBF], F32, name="b1t")
                nc.sync.dma_start(out=b1t[:, 0, :], in_=b1c[l, 0])
                nc.sync.dma_start(out=b1t[:, 1, :], in_=b1c[l, 1])
                b2t = gbp.tile([128, 2, DIM], F32, name="b2t")
                nc.sync.dma_start(out=b2t[:, 0, :], in_=b2b[l, 0])
                nc.sync.dma_start(out=b2t[:, 1, :], in_=b2b[l, 1])

                w1t = [wbig.tile([128, OBF, KB, 2, 128], FP8, name="w1t")
                       for _ in range(2)]
                for opl in range(2):
                    nc.sync.dma_start(out=w1t[opl][:], in_=w1[l, opl])
                w2t = [wbg2.tile([128, OBF, 2, DIM], FP8, name="w2t")
                       for _ in range(2)]
                for opl in range(2):
                    nc.sync.dma_start(out=w2t[opl][:], in_=w2[l, opl])

                ap2 = ap2p.tile([128, 2, OBF, TOK], FP8, name="ap2")
                with tc.tile_pool(name=un("psa"), bufs=4, space="PSUM") as psa:
                    for ob in range(OBF):
                        pA = []
                        for opl in range(2):
                            p_ = psa.tile([128, TOK], F32, name="pA")
                            for kb in range(KB):
                                nc.tensor.matmul(
                                    p_[:], w1t[opl][:, ob, kb, :, :],
                                    hT[:, :, kb, :],
                                    start=(kb == 0), stop=(kb == KB - 1),
                                    perf_mode=DR)
                            pA.append(p_)
                        # modrelu (psum a = WS1 x true, +bias)
                        a_r = ffp.tile([128, TOK], BF16, name="a_r")
                        a_i = ffp.tile([128, TOK], BF16, name="a_i")
                        nc.vector.tensor_scalar_add(a_r[:], pA[0][:],
                                                    b1t[:, 0, ob:ob + 1])
                        nc.vector.tensor_scalar_add(a_i[:], pA[1][:],
                                                    b1t[:, 1, ob:ob + 1])
                        sq_r = ffp.tile([128, TOK], BF16, name="sq_r")
                        sq_i = ffp.tile([128, TOK], BF16, name="sq_i")
                        nc.vector.tensor_mul(sq_r[:], a_r[:], a_r[:])
                        nc.vector.tensor_mul(sq_i[:], a_i[:], a_i[:])
                        m2 = ffp.tile([128, TOK], BF16, name="m2")
                        nc.vector.tensor_add(m2[:], sq_r[:], sq_i[:])
                        fac = ffp.tile([128, TOK], BF16, name="fac")
                        if zero_mb:
                            # ap_f8 = a_s * sqrt(m2*LAM^2/WS1^4) = LAM*a*mag
                            nc.scalar.activation(fac[:], m2[:], AF.Sqrt,
                                                 scale=LAM * LAM / WS1 ** 4)
                        else:
                            mag = ffp.tile([128, TOK], F32, name="mag")
                            nc.scalar.activation(mag[:], m2[:], AF.Sqrt,
                                                 scale=1.0 / (WS1 * WS1),
                                                 bias=tiny_t[:])
                            rel = ffp.tile([128, TOK], F32, name="rel")
                            nc.scalar.activation(rel[:], mag[:], AF.Relu,
                                                 bias=mb_t[:, l:l + 1])
                            rel2 = ffp.tile([128, TOK], F32, name="rel2")
                            nc.vector.scalar_tensor_tensor(
                                rel2[:], rel[:], LAM / WS1, rel[:],
                                op0=ALU.mult, op1=ALU.mult)
                            rmag = ffp.tile([128, TOK], F32, name="rmag")
                            nc.vector.reciprocal(rmag[:], mag[:])
                            nc.vector.tensor_mul(fac[:], rel2[:], rmag[:])
                        nc.vector.tensor_mul(ap2[:, 0, ob, :], a_r[:], fac[:])
                        nc.vector.tensor_mul(ap2[:, 1, ob, :], a_i[:], fac[:])

                # W2 (psum = 32*16 = 512x true) + bias + residual
                with tc.tile_pool(name=un("ps2"), bufs=4, space="PSUM") as ps2:
                    for opl in range(2):
                        for tb in range(TB):
                            p_ = ps2.tile([128, DIM], F32, name="p2")
                            for ob in range(OBF):
                                nc.tensor.matmul(
                                    p_[:],
                                    ap2[:, :, ob, tb * 128:(tb + 1) * 128],
                                    w2t[opl][:, ob, :, :],
                                    start=(ob == 0), stop=(ob == OBF - 1),
                                    perf_mode=DR)
                            nc.vector.scalar_tensor_tensor(
                                xc[opl][:, tb, :], p_[:],
                                1.0 / (LAM * WS),
                                xc[opl][:, tb, :],
                                op0=ALU.mult, op1=ALU.add)
                            nc.vector.tensor_add(xc[opl][:, tb, :],
                                                 xc[opl][:, tb, :],
                                                 b2t[:, opl, :])

            for l in range(L):
                attn_block(l, taps and l == 0)
                if taps and l == 0:
                    for p in range(2):
                        for tb in range(TB):
                            nc.sync.dma_start(out=tap_d["xc1"][p, tb],
                                              in_=xc[p][:, tb, :])
                ff_block(l, taps and l == 0)
                if taps and l == 0:
                    for p in range(2):
                        for tb in range(TB):
                            nc.sync.dma_start(out=tap_d["xc2"][p, tb],
                                              in_=xc[p][:, tb, :])

            # ---- final norm + output ----
            fo0 = ntmp.tile([128, TB, DIM], F32, name="fn0", bufs=1)
            fo1 = ntmp.tile([128, TB, DIM], F32, name="fn1", bufs=1)
            norm_natural(g_fin, [fo0, fo1], F32)
            for p_out, t in ((0, fo0), (1, fo1)):
                for tb in range(TB):
                    nc.sync.dma_start(out=out_d[p_out, tb], in_=t[:, tb, :])

    nc.compile()
    return nc


# ---------------------------------------------------------------------------
# host side: shard, run, unshard
# ---------------------------------------------------------------------------

FP8NP = ml_dtypes.float8_e4m3


def _prep_shared(Wq, Wkv, Wo, W1, b1, W2, b2, gamma_attn, gamma_ff, mod_bias,
                 gamma_final):
    """Host-side marshalling of the weight tensors (identical on all cores)."""
    def lp(w):  # [L, ..., 2] -> [L, 2, ...]
        return np.moveaxis(np.moveaxis(w, -1, 0), 0, 1)

    sh = {}
    wq_p = lp(Wq)                       # [L, 2, DIM, INNER]
    wk_p = lp(Wkv[:, :, :INNER, :])
    wv_p = lp(Wkv[:, :, INNER:, :])
    wo_p = lp(Wo)                       # [L, 2, INNER, DIM]
    w1_p = lp(W1)                       # [L, 2, DIM, FF]
    w2_p = lp(W2)                       # [L, 2, FF, DIM]

    def pair(w_p, scale):
        """[L, 2, K, M] -> term pairs [L, outpl, 2pair, K, M]."""
        Wr = w_p[:, 0] * scale
        Wi = w_p[:, 1] * scale
        return np.stack([np.stack([Wr, -Wi], axis=1),
                         np.stack([Wi, Wr], axis=1)], axis=1)

    def stationary(w_p, nob, scale, k_major=False):
        """-> [L, 2outpl, nob, 128k, KB, 2pair, 128m] (or k-major)."""
        P = pair(w_p, scale)                       # [L, 2, 2, K, M]
        P = P.reshape(L, 2, 2, KB, 128, nob, 128)  # [L,o,pr,kb,k,ob,m]
        if k_major:
            P = P.transpose(0, 1, 4, 5, 3, 2, 6)   # [L,o,k,ob,kb,pr,m]
        else:
            P = P.transpose(0, 1, 5, 4, 3, 2, 6)   # [L,o,ob,k,kb,pr,m]
        return np.ascontiguousarray(P).astype(FP8NP)

    def moving(w_p, scale, kc):
        """-> [L, 2outpl, 128k, kc, 2pair, OUT]."""
        P = pair(w_p, scale)                       # [L, 2, 2, K, OUT]
        P = P.reshape(L, 2, 2, kc, 128, -1)        # [L,o,pr,kc,k,out]
        P = P.transpose(0, 1, 4, 3, 2, 5)          # [L,o,k,kc,pr,out]
        return np.ascontiguousarray(P).astype(FP8NP)

    sh["wqk"] = np.ascontiguousarray(np.stack(
        [stationary(wq_p, 4, WS), stationary(wk_p, 4, WS)], axis=1))
    sh["w1"] = stationary(w1_p, OBF, WS1, k_major=True)
    sh["wv"] = moving(wv_p, WS, KB)
    sh["wo"] = moving(wo_p, WS, HP)
    sh["w2"] = moving(w2_p, WS, OBF)

    b1_p = lp(b1) * WS1                 # [L, 2, FF]
    sh["b1c"] = np.ascontiguousarray(
        b1_p.reshape(L, 2, OBF, 128).transpose(0, 1, 3, 2))
    b2_p = lp(b2)                       # [L, 2, DIM]
    sh["b2b"] = np.ascontiguousarray(
        np.broadcast_to(b2_p[:, :, None, :], (L, 2, 128, DIM)))
    ga = lp(gamma_attn)                 # [L, 2, DIM]
    sh["g_at"] = np.ascontiguousarray(
        np.broadcast_to(ga[:, :, None, :], (L, 2, 128, DIM)))
    gf = lp(gamma_ff)
    sh["g_ff"] = np.ascontiguousarray(
        np.broadcast_to(gf[:, :, None, :], (L, 2, 128, DIM)))
    gfin = np.moveaxis(gamma_final, -1, 0)      # [2, DIM]
    sh["g_fin"] = np.ascontiguousarray(
        np.broadcast_to(gfin[:, None, :], (2, 128, DIM)))
    sh["mbias"] = np.ascontiguousarray(
        np.broadcast_to(mod_bias[None, :], (128, L)).astype(np.float32))
    return sh


def _rot_tables(core):
    """cos/sin tables [128, TOK] for this core's token positions."""
    inv_freq = 1.0 / (10000.0 ** (np.arange(DH, dtype=np.float64) / DH))
    pos = (core % 4) * TOK + np.arange(TOK, dtype=np.float64)
    dh_idx = np.arange(128) % DH
    freqs = pos[None, :] * inv_freq[dh_idx][:, None]    # [128, TOK]
    return (np.cos(freqs).astype(np.float32),
            np.sin(freqs).astype(np.float32))


_NC_CACHE = {}


def get_nc(taps, unit_gamma, zero_mb):
    key = (taps, unit_gamma, zero_mb)
    if key not in _NC_CACHE:
        _NC_CACHE[key] = build_nc(taps=taps, unit_gamma=unit_gamma,
                                  zero_mb=zero_mb)
    return _NC_CACHE[key]


def make_in_maps(x, gamma_attn, Wq, Wkv, Wo, gamma_ff, W1, b1, mod_bias, W2,
                 b2, gamma_final):
    x = np.asarray(x, dtype=np.float32)
    sh = _prep_shared(np.asarray(Wq, np.float32), np.asarray(Wkv, np.float32),
                      np.asarray(Wo, np.float32), np.asarray(W1, np.float32),
                      np.asarray(b1, np.float32), np.asarray(W2, np.float32),
                      np.asarray(b2, np.float32),
                      np.asarray(gamma_attn, np.float32),
                      np.asarray(gamma_ff, np.float32),
                      np.asarray(mod_bias, np.float32),
                      np.asarray(gamma_final, np.float32))
    xf = x.reshape(B * N, DIM, 2)
    in_maps = []
    for core in range(NCORES):
        tok = xf[core * TOK:(core + 1) * TOK]           # [TOK, DIM, 2]
        xs = np.ascontiguousarray(
            tok.transpose(2, 0, 1).reshape(2, TB, 128, DIM))
        rc, rs = _rot_tables(core)
        m = dict(sh)
        m["x"] = xs
        m["rotc"] = rc
        m["rots"] = rs
        in_maps.append(m)
    return in_maps


def _flags(gamma_attn, gamma_ff, gamma_final, mod_bias):
    def unit(g):
        g = np.asarray(g, np.float32)
        return bool(np.all(g[..., 0] == 1.0) and np.all(g[..., 1] == 0.0))

    unit_gamma = unit(gamma_attn) and unit(gamma_ff) and unit(gamma_final)
    zero_mb = bool(np.all(np.asarray(mod_bias) == 0.0))
    return unit_gamma, zero_mb


def kernel(x, gamma_attn, Wq, Wkv, Wo, gamma_ff, W1, b1, mod_bias, W2, b2,
           gamma_final):
    unit_gamma, zero_mb = _flags(gamma_attn, gamma_ff, gamma_final, mod_bias)
    nc = get_nc(False, unit_gamma, zero_mb)
    in_maps = make_in_maps(x, gamma_attn, Wq, Wkv, Wo, gamma_ff, W1, b1,
                           mod_bias, W2, b2, gamma_final)
    res = run_bass_kernel_spmd(nc, in_maps, core_ids=list(range(NCORES)))
    outs = []
    for core in range(NCORES):
        o = res.results[core]["out"]                    # [2, TB, 128, DIM]
        o = o.reshape(2, TOK, DIM).transpose(1, 2, 0)   # [TOK, DIM, 2]
        outs.append(o)
    full = np.concatenate(outs, axis=0).reshape(B, N, DIM, 2)
    return np.ascontiguousarray(full.astype(np.float32))
